# revision 19
# baseline (speedup 1.0000x reference)
"""Trainium2 8-core GQA attention kernel (tensor-parallel over heads).

Strategy (8 NeuronCores, SPMD):
  - Core c owns q-heads [4c..4c+4) and kv-head c (GQA groups stay aligned).
  - Phases A (qkv projection + RoPE) and B (attention) are merged per token
    chunk: causality means chunk (b,qc) only attends k-chunks <= qc, so the
    attention for a chunk is emitted right after its projection and the Tile
    scheduler fills attention's exp-latency stalls with projection matmuls.
  - qkvT = wqkv_c^T @ x^T is computed feature-major so Q^T/K^T land in
    [head_dim, tokens] layout; RoPE applied with partition-shifted multiply-adds.
  - Attention scores are computed transposed (S^T[k,q]) so exp(S^T) feeds the
    PV matmul directly (lhsT = V[k,d]) with zero P transposes; fully-masked
    causal blocks are skipped; partially-masked blocks get a multiplicative
    {0,1} bf16 mask post-exp; denominators for all 4 heads accumulate into one
    [4, 512] PSUM row-set via indicator-column matmuls, 4 exp-blocks per
    matmul (summed on DVE); normalization is deferred to the output.
  - The AllGather of attention outputs is split into 8 token-chunk collectives
    issued as soon as each chunk's attention completes; phase C (the wo
    projection, out^T = wo_c^T @ attn^T) runs as a solid block at the end,
    by which time all AllGathers have long completed.
  - Host: shards/casts inputs, transposes x, concatenates output slices.
All PE math in bf16 (f32 PSUM accumulation).
"""

import numpy as np
import ml_dtypes

import concourse.bass as bass
import concourse.mybir as mybir
import concourse.tile as tile
from concourse import bacc
from concourse.bass_utils import run_bass_kernel_spmd

BF16 = mybir.dt.bfloat16
F32 = mybir.dt.float32
HD = 128            # head dim
HHD = HD // 2       # rope half
P = 128             # partitions
QCH = 512           # q-chunk / token-chunk size
KT = 128            # k tile (partition dim)
SCALE = 1.0 / np.sqrt(HD)


def build_graph(NB, S, D, HPC, NCORES, block_cls, n_mixed, qc_mask):
    """Build the per-core SPMD graph.

    block_cls[(qc, kt)] -> 'full' | 'skip' | int (mixed-mask slot index)
    qc_mask[qc] -> (first_slot, count) of that q-chunk's mixed-mask slots
    """
    TOK = NB * S
    QF = HPC * HD           # q features per core
    FLOC = QF + 2 * HD      # local qkv features (q + k + v)
    MT = FLOC // P          # feature tiles (q tiles + 1 k + 1 v)
    KD = D // P             # contraction tiles over model dim
    NQC = S // QCH          # q chunks per batch
    NKT = S // KT           # k tiles per batch
    KTC = QCH // KT         # k tiles per token chunk
    ODPC = D // NCORES      # output dims per core
    NCHK = TOK // QCH       # token chunks overall
    n_mask = max(n_mixed, 1)

    nc = bacc.Bacc("TRN2", target_bir_lowering=False, debug=False,
                   num_devices=NCORES)

    xt_d = nc.dram_tensor("xt", [D, TOK], BF16, kind="ExternalInput").ap()
    wqkv_d = nc.dram_tensor("wqkv", [D, FLOC], BF16, kind="ExternalInput").ap()
    wo_d = nc.dram_tensor("wo", [D, ODPC], BF16, kind="ExternalInput").ap()
    sc_d = nc.dram_tensor("sincos2", [P, 2 * S], BF16, kind="ExternalInput").ap()
    mask_d = nc.dram_tensor("maskblk", [n_mask * P, QCH], BF16,
                            kind="ExternalInput").ap()
    eye_d = nc.dram_tensor("eye", [HPC, HPC * P], BF16,
                           kind="ExternalInput").ap()
    out_d = nc.dram_tensor("out", [ODPC, TOK], F32, kind="ExternalOutput").ap()

    with tile.TileContext(nc) as tc:
        with tc.tile_pool(name="persist", bufs=1) as persist, \
             tc.tile_pool(name="dram", bufs=1, space="DRAM") as dram:
            qkvT = persist.tile([P, MT, TOK], BF16)
            v_kd = persist.tile([P, NB * NKT, HD], BF16)
            ident = persist.tile([P, P], BF16)
            nc.gpsimd.memset(ident[:], 0.0)
            nc.gpsimd.affine_select(
                out=ident[:], in_=ident[:],
                compare_op=mybir.AluOpType.not_equal, fill=1.0, base=0,
                pattern=[[-1, P]], channel_multiplier=1)
            # indicator columns/rows for per-head denominator batching
            ecol = persist.tile([P, HPC, HPC], BF16)   # [:, h, :] = e_h cols
            erow = persist.tile([HPC, HPC, P], BF16)   # [:, h, :] = e_h rows
            nc.vector.memset(ecol[:], 0.0)
            for h in range(HPC):
                nc.vector.memset(ecol[:, h, h:h + 1], 1.0)
            nc.scalar.dma_start(erow[:], eye_d[:])

            bounce = [dram.tile([QF, QCH], BF16, name=f"bnc{ci}")
                      for ci in range(NCHK)]
            agc = [dram.tile([QF * NCORES, QCH], BF16, name=f"agc{ci}",
                             addr_space="Shared" if NCORES > 4 else "Local")
                   for ci in range(NCHK)]

            # ---------- merged phases A (projection+RoPE) and B (attention) --
            with tc.tile_pool(name="pha", bufs=1) as pha, \
                 tc.tile_pool(name="phax", bufs=2) as phax, \
                 tc.tile_pool(name="phat", bufs=2) as phat, \
                 tc.tile_pool(name="phbw", bufs=3) as phbw, \
                 tc.tile_pool(name="phbm", bufs=2) as phbm, \
                 tc.tile_pool(name="psab", bufs=1, space="PSUM") as psab:
                KH = KD // 2
                wq_sb = pha.tile([P, KD, FLOC], BF16)

                def load_xt(col0, half, tagname):
                    xt_sb = phax.tile([P, KH, QCH], BF16, tag="xt",
                                      name=tagname)
                    nc.sync.dma_start(
                        xt_sb[:],
                        xt_d[half * KH * P:(half + 1) * KH * P,
                             col0:col0 + QCH]
                        .rearrange("(ko p) t -> p ko t", p=P))
                    return xt_sb

                for ko in range(2):
                    nc.sync.dma_start(
                        wq_sb[:, ko, :], wqkv_d[ko * P:(ko + 1) * P, :])
                xt_first = load_xt(0, 0, "xtf")
                for ko in range(2, KD // 2):
                    nc.sync.dma_start(
                        wq_sb[:, ko, :], wqkv_d[ko * P:(ko + 1) * P, :])
                xt_first1 = load_xt(0, 1, "xtf1")
                for ko in range(KD // 2, KD):
                    nc.sync.dma_start(
                        wq_sb[:, ko, :], wqkv_d[ko * P:(ko + 1) * P, :])
                sc_sb = pha.tile([P, 2 * S], BF16)
                nc.scalar.dma_start(sc_sb[:], sc_d[:])
                cosT = sc_sb[:, 0:S]
                sinT = sc_sb[:, S:2 * S]

                m_groups = [list(range(g, min(g + 3, MT)))
                            for g in range(0, MT, 3)]

                def proj_chunk(b, cb):
                    ch = b * (S // QCH) + cb
                    col0 = ch * QCH
                    s0 = col0 % S
                    for gi, grp in enumerate(m_groups):
                        pss = {m: psab.tile([P, QCH], F32, tag="pa", bufs=5,
                                            name=f"pa{ch}_{m}")
                               for m in grp}
                        for half in range(2):
                            if ch == 0 and gi == 0 and half == 0:
                                xt_sb = xt_first
                            elif ch == 0 and gi == 0 and half == 1:
                                xt_sb = xt_first1
                            else:
                                xt_sb = load_xt(col0, half,
                                                f"xt{ch}_{gi}_{half}")
                            for k in range(KH):
                                kg = half * KH + k
                                for m in grp:
                                    nc.tensor.matmul(
                                        pss[m][:],
                                        wq_sb[:, kg, m * P:(m + 1) * P],
                                        xt_sb[:, k, :],
                                        start=(kg == 0), stop=(kg == KD - 1))
                        for m in grp:
                            dst = qkvT[:, m, col0:col0 + QCH]
                            if m == MT - 1:  # v
                                nc.vector.tensor_copy(dst, pss[m][:])
                                continue
                            t1 = phat.tile([P, QCH], F32, tag="t1",
                                           name=f"t1_{ch}_{m}")
                            t2 = phat.tile([P, QCH], F32, tag="t2",
                                           name=f"t2_{ch}_{m}")
                            nc.vector.tensor_mul(t1[:], pss[m][:],
                                                 cosT[:, s0:s0 + QCH])
                            nc.vector.tensor_mul(t2[0:HHD, :],
                                                 pss[m][HHD:P, :],
                                                 sinT[0:HHD, s0:s0 + QCH])
                            nc.vector.tensor_mul(t2[HHD:P, :],
                                                 pss[m][0:HHD, :],
                                                 sinT[HHD:P, s0:s0 + QCH])
                            nc.vector.tensor_add(dst, t1[:], t2[:])
                    # V^T -> V via PE transposes for this chunk's k tiles
                    for kt in range(cb * KTC, (cb + 1) * KTC):
                        pt_ps = psab.tile([P, P], BF16, tag="pa", bufs=5,
                                          name=f"vt{b}_{kt}")
                        nc.tensor.transpose(
                            pt_ps[:],
                            qkvT[:, MT - 1,
                                 b * S + kt * KT:b * S + (kt + 1) * KT],
                            ident[:])
                        nc.vector.tensor_copy(v_kd[:, b * NKT + kt, :],
                                              pt_ps[:])

                def attn_chunk(ci, b, qc):
                    kts = [kt for kt in range(NKT)
                           if block_cls[(qc, kt)] != 'skip']
                    q0 = b * S + qc * QCH
                    mfirst, mcnt = qc_mask.get(qc, (0, 0))
                    if mcnt:
                        mk = phbm.tile([P, mcnt, QCH], BF16, tag="mk",
                                       name=f"mk{ci}")
                        nc.scalar.dma_start(
                            mk[:],
                            mask_d[mfirst * P:(mfirst + mcnt) * P, :]
                            .rearrange("(mb p) q -> p mb q", p=P))
                    d_ps = psab.tile([HPC, QCH], F32, tag="pa", bufs=5,
                                     name=f"den{ci}")
                    o_tiles = {}
                    for h in range(HPC):
                        o_ps = psab.tile([P, QCH], F32, tag="outT", bufs=1,
                                         name=f"o{ci}_{h}")
                        for i, kt in enumerate(kts):
                            st = psab.tile([P, QCH], F32, tag="st", bufs=2,
                                           name=f"st{ci}_{h}_{i}")
                            nc.tensor.matmul(
                                st[:],
                                qkvT[:, HPC,
                                     b * S + kt * KT:b * S + (kt + 1) * KT],
                                qkvT[:, h, q0:q0 + QCH],
                                start=True, stop=True)
                            pt = phbw.tile([P, QCH], BF16, tag="pt", bufs=5,
                                           name=f"pt{ci}_{h}_{i}")
                            nc.scalar.activation(
                                pt[:], st[:],
                                mybir.ActivationFunctionType.Exp,
                                bias=0.0, scale=float(SCALE))
                            cls = block_cls[(qc, kt)]
                            if cls != 'full':
                                nc.vector.tensor_mul(pt[:], pt[:],
                                                     mk[:, cls - mfirst, :])
                            first, last = (i == 0), (i == len(kts) - 1)
                            nc.tensor.matmul(
                                o_ps[:], v_kd[:, b * NKT + kt, :], pt[:],
                                start=first, stop=last)
                            # group up to 4 exp blocks per denominator matmul
                            gpos = i % 4
                            if gpos == 0:
                                dacc, dacc_n = pt, 1
                            else:
                                if dacc_n == 1:
                                    dsum = phbw.tile([P, QCH], BF16,
                                                     tag="dsum", bufs=2,
                                                     name=f"ds{ci}_{h}_{i}")
                                    nc.vector.tensor_add(dsum[:], dacc[:],
                                                         pt[:])
                                    dacc = dsum
                                else:
                                    nc.vector.tensor_add(dacc[:], dacc[:],
                                                         pt[:])
                                dacc_n += 1
                            if gpos == 3 or last:
                                nc.tensor.matmul(
                                    d_ps[:], ecol[:, h, :], dacc[:],
                                    start=(i < 4 and h == 0),
                                    stop=(last and h == HPC - 1))
                        o_sb = phbw.tile([P, QCH], BF16, tag="osbuf", bufs=4,
                                         name=f"ou{ci}_{h}")
                        nc.vector.tensor_copy(o_sb[:], o_ps[:])
                        o_tiles[h] = o_sb
                    inv = phbw.tile([HPC, QCH], F32, tag="inv",
                                    name=f"inv{ci}")
                    nc.vector.reciprocal(inv[:], d_ps[:])
                    invb = phbw.tile([HPC, QCH], BF16, tag="invb",
                                     name=f"invb{ci}")
                    nc.vector.tensor_copy(invb[:], inv[:])
                    for h in range(HPC):
                        bc_ps = psab.tile([P, QCH], F32, tag="st", bufs=2,
                                          name=f"bc{ci}_{h}")
                        nc.tensor.matmul(bc_ps[:], erow[:, h, :], invb[:],
                                         start=True, stop=True)
                        bcc = phbw.tile([P, QCH], BF16, tag="bcc", bufs=2,
                                        name=f"bcc{ci}_{h}")
                        nc.vector.tensor_copy(bcc[:], bc_ps[:])
                        at = phbw.tile([P, QCH], BF16, tag="at", bufs=2,
                                       name=f"at{ci}_{h}")
                        nc.vector.tensor_mul(at[:], o_tiles[h][:], bcc[:])
                        nc.scalar.dma_start(
                            bounce[ci][h * P:(h + 1) * P, :], at[:])

                # a q-chunk's attention can only run once every k-chunk it
                # attends is projected (for causal masks: its own chunk)
                def max_kchunk(qc):
                    kts = [kt for kt in range(NKT)
                           if block_cls[(qc, kt)] != 'skip']
                    return (max(kts) // KTC) if kts else 0

                for b in range(NB):
                    done = set()
                    for cb in range(S // QCH):
                        proj_chunk(b, cb)
                        for qc in range(NQC):
                            if qc in done or max_kchunk(qc) > cb:
                                continue
                            done.add(qc)
                            ci = b * NQC + qc
                            attn_chunk(ci, b, qc)
                            nc.gpsimd.collective_compute(
                                "AllGather", mybir.AluOpType.bypass,
                                replica_groups=[list(range(NCORES))],
                                ins=[bounce[ci].opt()], outs=[agc[ci].opt()])

            # ---------------- Phase C: out^T = wo_c^T @ attn^T ---------------
            with tc.tile_pool(name="phc", bufs=1) as phc, \
                 tc.tile_pool(name="phcx", bufs=2) as phcx, \
                 tc.tile_pool(name="phco", bufs=2) as phco, \
                 tc.tile_pool(name="psc", bufs=1, space="PSUM") as psc:
                wo_sb = phc.tile([P, KD, ODPC], BF16)
                for ko in range(KD):
                    eng = nc.sync if ko % 2 == 0 else nc.scalar
                    eng.dma_start(
                        wo_sb[:, ko, :], wo_d[ko * P:(ko + 1) * P, :])
                for ci in range(NCHK):
                    tok0 = ci * QCH
                    agt = phcx.tile([P, KD, QCH], BF16, tag="agt",
                                    name=f"agt{ci}")
                    kh2 = KD // 2
                    for half in range(2):
                        nc.sync.dma_start(
                            agt[:, half * kh2:(half + 1) * kh2, :],
                            agc[ci][half * kh2 * P:(half + 1) * kh2 * P, :]
                            .rearrange("(ko p) t -> p ko t", p=P))
                    for md in range(ODPC // P):
                        po = psc.tile([P, QCH], F32, tag="po", bufs=4,
                                      name=f"po{ci}_{md}")
                        for kf in range(KD):
                            nc.tensor.matmul(
                                po[:],
                                wo_sb[:, kf, md * P:(md + 1) * P],
                                agt[:, kf, :],
                                start=(kf == 0), stop=(kf == KD - 1))
                        osb = phco.tile([P, QCH], F32, tag="osb",
                                        name=f"osb{ci}_{md}")
                        nc.vector.tensor_copy(osb[:], po[:])
                        nc.sync.dma_start(
                            out_d[md * P:(md + 1) * P,
                                  tok0:tok0 + QCH], osb[:])

    nc.compile()
    return nc


def _host_prep(x, wqkv, wo, sincos, full_causal_mask, start_pos,
               NB, S, D, HPC, NCORES):
    """Shard, cast, and lay out inputs; classify mask blocks."""
    bf16 = ml_dtypes.bfloat16
    TOK = NB * S
    H = HPC * NCORES
    QF = HPC * HD
    NQC = S // QCH
    NKT = S // KT
    ODPC = D // NCORES
    q_sz = H * HD

    xt = np.ascontiguousarray(x.reshape(TOK, D).T).astype(bf16)

    # effective mask: [q, k] (batch-shared), incl. the cache-validity term
    m_eff = np.asarray(full_causal_mask[0, 0], dtype=bool)
    m_eff = m_eff[start_pos:start_pos + S, :S].copy()
    valid = np.arange(S) < (start_pos + S)
    m_eff &= valid[None, :]

    block_cls = {}
    mixed_blocks = []
    qc_mask = {}
    for qc in range(NQC):
        first = len(mixed_blocks)
        for kt in range(NKT):
            blk = m_eff[qc * QCH:(qc + 1) * QCH, kt * KT:(kt + 1) * KT]
            if blk.all():
                block_cls[(qc, kt)] = 'full'
            elif not blk.any():
                block_cls[(qc, kt)] = 'skip'
            else:
                block_cls[(qc, kt)] = len(mixed_blocks)
                mixed_blocks.append(
                    np.ascontiguousarray(blk.T.astype(np.float32)))  # [k, q]
        cnt = len(mixed_blocks) - first
        if cnt:
            qc_mask[qc] = (first, cnt)
    n_mixed = len(mixed_blocks)
    if n_mixed:
        maskblk = np.concatenate(mixed_blocks, axis=0).astype(bf16)
    else:
        maskblk = np.zeros((P, QCH), dtype=bf16)

    # rope tables, transposed + duplicated halves; sin rows 0:64 negated
    sc = np.asarray(sincos[start_pos:start_pos + S], dtype=np.float32)
    sin, cos = sc[:, :HHD], sc[:, HHD:]
    cosT2 = np.concatenate([cos.T, cos.T], axis=0)           # [128, S]
    sinT2 = np.concatenate([-sin.T, sin.T], axis=0)          # [128, S]
    sincos2 = np.concatenate([cosT2, sinT2], axis=1).astype(bf16)

    eye = np.zeros((HPC, HPC, P), dtype=bf16)
    for h in range(HPC):
        eye[h, h, :] = 1
    eye = eye.reshape(HPC, HPC * P)

    in_maps = []
    for c in range(NCORES):
        qcols = np.asarray(wqkv[:, c * QF:(c + 1) * QF])
        kcols = np.asarray(wqkv[:, q_sz + c * HD:q_sz + (c + 1) * HD])
        vcols = np.asarray(
            wqkv[:, q_sz + NCORES * HD + c * HD:
                 q_sz + NCORES * HD + (c + 1) * HD])
        wqkv_c = np.concatenate([qcols, kcols, vcols], axis=1).astype(bf16)
        wo_c = np.ascontiguousarray(
            np.asarray(wo[:, c * ODPC:(c + 1) * ODPC])).astype(bf16)
        in_maps.append({
            "xt": xt, "wqkv": wqkv_c, "wo": wo_c,
            "sincos2": sincos2, "maskblk": maskblk, "eye": eye,
        })
    return in_maps, block_cls, n_mixed, qc_mask


_CACHE = {}


def run_distributed(x, wqkv, wo, sincos, full_causal_mask, start_pos,
                    NB, S, D, HPC, NCORES, trace=False, tmpdir=None):
    in_maps, block_cls, n_mixed, qc_mask = _host_prep(
        x, wqkv, wo, sincos, full_causal_mask, start_pos,
        NB, S, D, HPC, NCORES)
    key = (NB, S, D, HPC, NCORES,
           tuple(sorted((k, v) for k, v in block_cls.items())))
    if key not in _CACHE:
        _CACHE[key] = build_graph(NB, S, D, HPC, NCORES, block_cls, n_mixed,
                                  qc_mask)
    nc = _CACHE[key]
    res = run_bass_kernel_spmd(nc, in_maps, list(range(NCORES)), trace=trace,
                               tmpdir=tmpdir)
    TOK = NB * S
    out = np.empty((TOK, D), dtype=np.float32)
    ODPC = D // NCORES
    for c in range(NCORES):
        out[:, c * ODPC:(c + 1) * ODPC] = res.results[c]["out"].T
    return out.reshape(NB, S, D), res


def kernel(x, wqkv, wo, sincos, cache_k, cache_v, full_causal_mask,
           start_pos) -> np.ndarray:
    x = np.asarray(x)
    start_pos = int(np.asarray(start_pos))
    B, S_, D_ = x.shape
    assert start_pos == 0, "prefill-only kernel (seq fills the whole cache)"
    out, _ = run_distributed(
        x, np.asarray(wqkv), np.asarray(wo), np.asarray(sincos),
        np.asarray(full_causal_mask), start_pos,
        NB=B, S=S_, D=D_, HPC=4, NCORES=8)
    return out
